# revision 2
# baseline (speedup 1.0000x reference)
"""Segmented irrep linear (128x0e+128x1o+128x2e) on 8 TRN2 NeuronCores.

y[n, off_l + u*d_l + i] = pw * sum_u' x[n, off_l + u'*d_l + i] * W_l[u', u]  (+ b on l=0)

Data-parallel over nodes: 100000 nodes padded to 8 * 12544, one shard per core.
Per 128-node tile on device:
  - 9 PE transposes turn the node-major x tile into per-(l,i) [u, n] blocks
  - DVE copies PSUM -> SBUF
  - 10 fp32 matmuls (incl. a K=1 ones x bias matmul fused into segment 0's
    accumulation group) produce [n, v] blocks in PSUM
  - ACT copies de-interleave PSUM -> SBUF output tile
  - contiguous DMAs both directions (4608B runs)
Weight pre-scaled by 128^-0.5 on host; bias appended via the K=1 matmul.
"""

import numpy as np

import concourse.bass as bass
import concourse.tile as tile
from concourse import bacc, mybir
from concourse.bass_utils import run_bass_kernel_spmd

N_CORES = 8
N_NODES = 100000
DIM = 1152
IRREPS = [(128, 1), (128, 3), (128, 5)]
SEG_OFF_X = [0, 128, 512]
PW = 1.0 / np.sqrt(128.0)

TILE_P = 128
TILES_PER_CORE = 98
SHARD = TILES_PER_CORE * TILE_P  # 12544
PAD_NODES = N_CORES * SHARD  # 100352
MACRO = 2  # node-tiles per DMA transfer (2 * 0.59MB = 1.18MB)

_cache = {}


def _build(shard_nodes=SHARD, macro=MACRO):
    assert shard_nodes % (TILE_P * macro) == 0
    n_macros = shard_nodes // (TILE_P * macro)

    nc = bacc.Bacc(
        "TRN2", target_bir_lowering=False, debug=False, num_devices=N_CORES
    )
    f32 = mybir.dt.float32
    x_d = nc.dram_tensor("x", [shard_nodes, DIM], f32, kind="ExternalInput")
    w_d = nc.dram_tensor("w", [128, 384], f32, kind="ExternalInput")
    aux_d = nc.dram_tensor("aux", [1, 256], f32, kind="ExternalInput")
    id_d = nc.dram_tensor("ident", [128, 128], f32, kind="ExternalInput")
    y_d = nc.dram_tensor("y", [shard_nodes, DIM], f32, kind="ExternalOutput")

    # [node, feat] -> [p, t, feat] with node = t*128 + p
    x_v = x_d.ap().rearrange("(t p) f -> p t f", p=TILE_P)
    y_v = y_d.ap().rearrange("(t p) f -> p t f", p=TILE_P)

    with tile.TileContext(nc) as tc:
        with (
            tc.tile_pool(name="const", bufs=1) as const_pool,
            tc.tile_pool(name="xin", bufs=3) as x_pool,
            tc.tile_pool(name="xt", bufs=3) as xt_pool,
            tc.tile_pool(name="out", bufs=3) as out_pool,
            tc.tile_pool(name="psT", bufs=2, space=bass.MemorySpace.PSUM) as psT_pool,
            tc.tile_pool(name="psO", bufs=2, space=bass.MemorySpace.PSUM) as psO_pool,
        ):
            w_sb = const_pool.tile([128, 384], f32)
            nc.sync.dma_start(w_sb[:], w_d.ap())
            aux_sb = const_pool.tile([1, 256], f32)
            nc.sync.dma_start(aux_sb[:], aux_d.ap())
            id_sb = const_pool.tile([128, 128], f32)
            nc.sync.dma_start(id_sb[:], id_d.ap())

            for t in range(n_macros):
                x_sb = x_pool.tile([TILE_P, macro, DIM], f32)
                nc.sync.dma_start(x_sb[:], x_v[:, t * macro:(t + 1) * macro, :])

                out_sb = out_pool.tile([TILE_P, macro, DIM], f32)

                for j in range(macro):
                    for l, (mul, d) in enumerate(IRREPS):
                        off = SEG_OFF_X[l]
                        # [p, u*d+i] -> [p, i, u] (stride-d slices per i)
                        seg = x_sb[:, j, off:off + mul * d].rearrange(
                            "p (u d) -> p d u", d=d
                        )
                        psT = psT_pool.tile([128, d * 128], f32, tag="psT")
                        for i in range(d):
                            nc.tensor.transpose(
                                psT[:, i * 128:(i + 1) * 128], seg[:, i, :], id_sb[:]
                            )
                        xt_sb = xt_pool.tile([128, d * 128], f32, tag="xt")
                        nc.vector.tensor_copy(xt_sb[:], psT[:])

                        psO = psO_pool.tile([128, d * 128], f32, tag="psO")
                        for i in range(d):
                            o_sl = psO[:, i * 128:(i + 1) * 128]
                            if l == 0:
                                # bias via ones[1,128].T @ b[1,128]
                                nc.tensor.matmul(
                                    o_sl, aux_sb[:1, 0:128], aux_sb[:1, 128:256],
                                    start=True, stop=False,
                                )
                            nc.tensor.matmul(
                                o_sl,
                                xt_sb[:, i * 128:(i + 1) * 128],
                                w_sb[:, l * 128:(l + 1) * 128],
                                start=(l != 0), stop=True,
                            )
                        # psum [p, i, v] -> out[p, j, off + v*d + i]
                        dst = out_sb[:, j, off:off + mul * d].rearrange(
                            "p (v d) -> p d v", d=d
                        )
                        src = psO[:].rearrange("p (i v) -> p i v", v=128)
                        nc.scalar.copy(dst, src)

                nc.sync.dma_start(y_v[:, t * macro:(t + 1) * macro, :], out_sb[:])

    nc.compile()
    return nc


def _get_nc():
    if "nc" not in _cache:
        _cache["nc"] = _build()
    return _cache["nc"]


def _host_prep(w, b):
    w = np.asarray(w, dtype=np.float32)
    b = np.asarray(b, dtype=np.float32)
    w_pack = np.empty((128, 384), dtype=np.float32)
    off = 0
    for l, (mul, d) in enumerate(IRREPS):
        W = w[off:off + mul * mul].reshape(mul, mul)  # [u, v]
        w_pack[:, l * 128:(l + 1) * 128] = PW * W
        off += mul * mul
    aux = np.empty((1, 256), dtype=np.float32)
    aux[0, :128] = 1.0
    aux[0, 128:] = b
    ident = np.eye(128, dtype=np.float32)
    return w_pack, aux, ident


def _ensure_ntff_hook():
    """The agent image's antenv lacks axon_hooks; synthesize it from the
    boot package's ctypes NTFF hook so trace=True works."""
    import sys
    import types

    if "antenv.axon_hooks" in sys.modules:
        return
    try:
        from trn_agent_boot.trn_boot import _ntff_profile_via_ctypes

        hook = _ntff_profile_via_ctypes("/opt/axon/libaxon_pjrt.so")
    except Exception:
        hook = None
    mod = types.ModuleType("antenv.axon_hooks")
    state = {"hook": hook}
    mod.get_axon_ntff_profile_hook = lambda: state["hook"]
    mod.set_axon_ntff_profile_hook = lambda h: state.__setitem__("hook", h)
    sys.modules["antenv.axon_hooks"] = mod
    import antenv

    antenv.axon_hooks = mod


def kernel(x, w, b, *, trace=False, trace_cores=None):
    if trace:
        _ensure_ntff_hook()
    x = np.ascontiguousarray(np.asarray(x, dtype=np.float32))
    assert x.shape == (N_NODES, DIM)
    w_pack, aux, ident = _host_prep(w, b)

    x_pad = np.zeros((PAD_NODES, DIM), dtype=np.float32)
    x_pad[:N_NODES] = x

    in_maps = [
        {
            "x": x_pad[c * SHARD:(c + 1) * SHARD],
            "w": w_pack,
            "aux": aux,
            "ident": ident,
        }
        for c in range(N_CORES)
    ]
    nc = _get_nc()
    res = run_bass_kernel_spmd(
        nc, in_maps, list(range(N_CORES)), trace=trace, trace_cores=trace_cores
    )
    _cache["last_result"] = res
    y = np.concatenate([res.results[c]["y"] for c in range(N_CORES)], axis=0)
    return y[:N_NODES]


# revision 3
# speedup vs baseline: 1.0805x; 1.0805x over previous
"""Segmented irrep linear (128x0e+128x1o+128x2e) on 8 TRN2 NeuronCores.

y[n, off_l + u*d_l + i] = pw * sum_u' x[n, off_l + u'*d_l + i] * W_l[u', u]  (+ b on l=0)

Data-parallel over nodes: 100000 nodes padded to 8 * 12544, one shard per core.
Per 128-node tile on device:
  - 9 PE transposes turn the node-major x tile into per-(l,i) [u, n] blocks
  - ACT copies PSUM -> SBUF (contiguous)
  - 9 fp32 matmuls produce [n, v] blocks in PSUM
  - DVE copies de-interleave PSUM -> SBUF output tile (seg 0's copy is a
    tensor_tensor add that applies the bias from a broadcast tile)
  - contiguous DMAs both directions (4608B runs)
Weight pre-scaled by 128^-0.5 on host.
"""

import numpy as np

import concourse.bass as bass
import concourse.tile as tile
from concourse import bacc, mybir
from concourse.bass_utils import run_bass_kernel_spmd

N_CORES = 8
N_NODES = 100000
DIM = 1152
IRREPS = [(128, 1), (128, 3), (128, 5)]
SEG_OFF_X = [0, 128, 512]
PW = 1.0 / np.sqrt(128.0)

TILE_P = 128
TILES_PER_CORE = 98
SHARD = TILES_PER_CORE * TILE_P  # 12544
PAD_NODES = N_CORES * SHARD  # 100352
MACRO = 4  # node-tiles per DMA transfer; last macro may be smaller

_cache = {}


def _build(shard_nodes=SHARD, macro=MACRO):
    n_tiles = shard_nodes // TILE_P
    assert shard_nodes % TILE_P == 0

    nc = bacc.Bacc(
        "TRN2", target_bir_lowering=False, debug=False, num_devices=N_CORES
    )
    f32 = mybir.dt.float32
    x_d = nc.dram_tensor("x", [shard_nodes, DIM], f32, kind="ExternalInput")
    w_d = nc.dram_tensor("w", [128, 384], f32, kind="ExternalInput")
    bias_d = nc.dram_tensor("bias", [128, 128], f32, kind="ExternalInput")
    id_d = nc.dram_tensor("ident", [128, 128], f32, kind="ExternalInput")
    y_d = nc.dram_tensor("y", [shard_nodes, DIM], f32, kind="ExternalOutput")

    # [node, feat] -> [p, t, feat] with node = t*128 + p
    x_v = x_d.ap().rearrange("(t p) f -> p t f", p=TILE_P)
    y_v = y_d.ap().rearrange("(t p) f -> p t f", p=TILE_P)

    with tile.TileContext(nc) as tc:
        with (
            tc.tile_pool(name="const", bufs=1) as const_pool,
            tc.tile_pool(name="xin", bufs=3) as x_pool,
            tc.tile_pool(name="xt", bufs=3) as xt_pool,
            tc.tile_pool(name="out", bufs=3) as out_pool,
            tc.tile_pool(name="psT", bufs=2, space=bass.MemorySpace.PSUM) as psT_pool,
            tc.tile_pool(name="psO", bufs=2, space=bass.MemorySpace.PSUM) as psO_pool,
        ):
            w_sb = const_pool.tile([128, 384], f32)
            nc.sync.dma_start(w_sb[:], w_d.ap())
            bias_sb = const_pool.tile([128, 128], f32)
            nc.sync.dma_start(bias_sb[:], bias_d.ap())
            id_sb = const_pool.tile([128, 128], f32)
            nc.sync.dma_start(id_sb[:], id_d.ap())

            t0 = 0
            while t0 < n_tiles:
                m = min(macro, n_tiles - t0)
                x_sb = x_pool.tile([TILE_P, macro, DIM], f32, tag="x")
                nc.sync.dma_start(
                    x_sb[:, :m, :], x_v[:, t0:t0 + m, :]
                )
                out_sb = out_pool.tile([TILE_P, macro, DIM], f32, tag="out")

                for j in range(m):
                    for l, (mul, d) in enumerate(IRREPS):
                        off = SEG_OFF_X[l]
                        # [p, u*d+i] -> [p, i, u] (stride-d slices per i)
                        seg = x_sb[:, j, off:off + mul * d].rearrange(
                            "p (u d) -> p d u", d=d
                        )
                        psT = psT_pool.tile([128, d * 128], f32, tag="psT")
                        for i in range(d):
                            nc.tensor.transpose(
                                psT[:, i * 128:(i + 1) * 128], seg[:, i, :], id_sb[:]
                            )
                        xt_sb = xt_pool.tile([128, d * 128], f32, tag="xt")
                        nc.scalar.copy(xt_sb[:], psT[:])

                        psO = psO_pool.tile([128, d * 128], f32, tag="psO")
                        for i in range(d):
                            nc.tensor.matmul(
                                psO[:, i * 128:(i + 1) * 128],
                                xt_sb[:, i * 128:(i + 1) * 128],
                                w_sb[:, l * 128:(l + 1) * 128],
                                start=True, stop=True,
                            )
                        # psum [p, i, v] -> out[p, j, off + v*d + i]
                        if l == 0:
                            nc.vector.tensor_add(
                                out_sb[:, j, 0:128], psO[:], bias_sb[:]
                            )
                        else:
                            dst = out_sb[:, j, off:off + mul * d].rearrange(
                                "p (v d) -> p d v", d=d
                            )
                            src = psO[:].rearrange("p (i v) -> p i v", v=128)
                            nc.vector.tensor_copy(dst, src)

                nc.sync.dma_start(
                    y_v[:, t0:t0 + m, :], out_sb[:, :m, :]
                )
                t0 += m

    nc.compile()
    return nc


def _get_nc():
    if "nc" not in _cache:
        _cache["nc"] = _build()
    return _cache["nc"]


def _host_prep(w, b):
    w = np.asarray(w, dtype=np.float32)
    b = np.asarray(b, dtype=np.float32)
    w_pack = np.empty((128, 384), dtype=np.float32)
    off = 0
    for l, (mul, d) in enumerate(IRREPS):
        W = w[off:off + mul * mul].reshape(mul, mul)  # [u, v]
        w_pack[:, l * 128:(l + 1) * 128] = PW * W
        off += mul * mul
    bias_bcast = np.broadcast_to(b[None, :], (128, 128)).copy()
    ident = np.eye(128, dtype=np.float32)
    return w_pack, bias_bcast, ident


def _ensure_ntff_hook():
    """The agent image's antenv lacks axon_hooks; synthesize it from the
    boot package's ctypes NTFF hook so trace=True works."""
    import sys
    import types

    if "antenv.axon_hooks" in sys.modules:
        return
    try:
        from trn_agent_boot.trn_boot import _ntff_profile_via_ctypes

        hook = _ntff_profile_via_ctypes("/opt/axon/libaxon_pjrt.so")
    except Exception:
        hook = None
    mod = types.ModuleType("antenv.axon_hooks")
    state = {"hook": hook}
    mod.get_axon_ntff_profile_hook = lambda: state["hook"]
    mod.set_axon_ntff_profile_hook = lambda h: state.__setitem__("hook", h)
    sys.modules["antenv.axon_hooks"] = mod
    import antenv

    antenv.axon_hooks = mod


def kernel(x, w, b, *, trace=False, trace_cores=None):
    if trace:
        _ensure_ntff_hook()
    x = np.ascontiguousarray(np.asarray(x, dtype=np.float32))
    assert x.shape == (N_NODES, DIM)
    w_pack, bias_bcast, ident = _host_prep(w, b)

    x_pad = np.zeros((PAD_NODES, DIM), dtype=np.float32)
    x_pad[:N_NODES] = x

    in_maps = [
        {
            "x": x_pad[c * SHARD:(c + 1) * SHARD],
            "w": w_pack,
            "bias": bias_bcast,
            "ident": ident,
        }
        for c in range(N_CORES)
    ]
    nc = _get_nc()
    res = run_bass_kernel_spmd(
        nc, in_maps, list(range(N_CORES)), trace=trace, trace_cores=trace_cores
    )
    _cache["last_result"] = res
    y = np.concatenate([res.results[c]["y"] for c in range(N_CORES)], axis=0)
    return y[:N_NODES]


# revision 5
# speedup vs baseline: 1.3789x; 1.2762x over previous
"""Segmented irrep linear (128x0e+128x1o+128x2e) on 8 TRN2 NeuronCores.

y[n, off_l + u*d_l + i] = pw * sum_u' x[n, off_l + u'*d_l + i] * W_l[u', u]  (+ b on l=0)

Data-parallel over nodes: 100000 nodes padded to 8 * 12544, one shard per core.
Per 128-node tile on device:
  - 9 PE transposes turn the node-major x tile into per-(l,i) [u, n] blocks
  - ACT copies PSUM -> SBUF (contiguous)
  - 9 fp32 matmuls produce [n, v] blocks in PSUM
  - DVE copies de-interleave PSUM -> SBUF output tile (seg 0's copy is a
    tensor_tensor add that applies the bias from a broadcast tile)
  - contiguous DMAs both directions (4608B runs)
Weight pre-scaled by 128^-0.5 on host.
"""

import numpy as np

import concourse.bass as bass
import concourse.tile as tile
from concourse import bacc, mybir
from concourse.bass_utils import run_bass_kernel_spmd

N_CORES = 8
N_NODES = 100000
DIM = 1152
IRREPS = [(128, 1), (128, 3), (128, 5)]
SEG_OFF_X = [0, 128, 512]
PW = 1.0 / np.sqrt(128.0)

TILE_P = 128
TILES_PER_CORE = 98
SHARD = TILES_PER_CORE * TILE_P  # 12544
PAD_NODES = N_CORES * SHARD  # 100352
MACRO = 4  # node-tiles per DMA transfer; last macro may be smaller

_cache = {}


def _build(shard_nodes=SHARD, macro=MACRO):
    n_tiles = shard_nodes // TILE_P
    assert shard_nodes % TILE_P == 0

    nc = bacc.Bacc(
        "TRN2", target_bir_lowering=False, debug=False, num_devices=N_CORES
    )
    f32 = mybir.dt.float32
    x_d = nc.dram_tensor("x", [shard_nodes, DIM], f32, kind="ExternalInput")
    w_d = nc.dram_tensor("w", [128, 384], f32, kind="ExternalInput")
    bias_d = nc.dram_tensor("bias", [128, 128], f32, kind="ExternalInput")
    id_d = nc.dram_tensor("ident", [128, 128], f32, kind="ExternalInput")
    y_d = nc.dram_tensor("y", [shard_nodes, DIM], f32, kind="ExternalOutput")

    # [node, feat] -> [p, t, feat] with node = t*128 + p
    x_v = x_d.ap().rearrange("(t p) f -> p t f", p=TILE_P)
    y_v = y_d.ap().rearrange("(t p) f -> p t f", p=TILE_P)

    with tile.TileContext(nc) as tc:
        with (
            tc.tile_pool(name="const", bufs=1) as const_pool,
            tc.tile_pool(name="xin", bufs=3) as x_pool,
            tc.tile_pool(name="xt", bufs=6) as xt_pool,
            tc.tile_pool(name="out", bufs=3) as out_pool,
            tc.tile_pool(name="psT", bufs=2, space=bass.MemorySpace.PSUM) as psT_pool,
            tc.tile_pool(name="psO", bufs=2, space=bass.MemorySpace.PSUM) as psO_pool,
        ):
            id_sb = const_pool.tile([128, 128], f32)
            nc.sync.dma_start(id_sb[:], id_d.ap())
            w_sb = const_pool.tile([128, 384], f32)
            nc.sync.dma_start(w_sb[:], w_d.ap())
            bias_sb = const_pool.tile([128, 128], f32)
            nc.sync.dma_start(bias_sb[:], bias_d.ap())

            # ramp: small first transfers so compute starts early
            sizes = []
            rem = n_tiles
            for s in (1, 1, 2):
                if rem > macro + s:
                    sizes.append(s)
                    rem -= s
            while rem > 0:
                m = min(macro, rem)
                sizes.append(m)
                rem -= m

            t0 = 0
            for m in sizes:
                x_sb = x_pool.tile([TILE_P, macro, DIM], f32, tag="x")
                nc.sync.dma_start(
                    x_sb[:, :m, :], x_v[:, t0:t0 + m, :]
                )
                out_sb = out_pool.tile([TILE_P, macro, DIM], f32, tag="out")

                for j in range(m):
                    for l, (mul, d) in enumerate(IRREPS):
                        off = SEG_OFF_X[l]
                        # [p, u*d+i] -> [p, i, u] (stride-d slices per i)
                        seg = x_sb[:, j, off:off + mul * d].rearrange(
                            "p (u d) -> p d u", d=d
                        )
                        psT = psT_pool.tile([128, d * 128], f32, tag="psT")
                        for i in range(d):
                            nc.tensor.transpose(
                                psT[:, i * 128:(i + 1) * 128], seg[:, i, :], id_sb[:]
                            )
                        xt_sb = xt_pool.tile([128, d * 128], f32, tag="xt")
                        nc.scalar.copy(xt_sb[:], psT[:])

                        psO = psO_pool.tile([128, d * 128], f32, tag="psO")
                        for i in range(d):
                            nc.tensor.matmul(
                                psO[:, i * 128:(i + 1) * 128],
                                xt_sb[:, i * 128:(i + 1) * 128],
                                w_sb[:, l * 128:(l + 1) * 128],
                                start=True, stop=True,
                            )
                        # psum [p, i, v] -> out[p, j, off + v*d + i]
                        if l == 0:
                            nc.vector.tensor_add(
                                out_sb[:, j, 0:128], psO[:], bias_sb[:]
                            )
                        else:
                            dst = out_sb[:, j, off:off + mul * d].rearrange(
                                "p (v d) -> p d v", d=d
                            )
                            src = psO[:].rearrange("p (i v) -> p i v", v=128)
                            nc.vector.tensor_copy(dst, src)

                nc.sync.dma_start(
                    y_v[:, t0:t0 + m, :], out_sb[:, :m, :]
                )
                t0 += m

    nc.compile()
    return nc


def _get_nc():
    if "nc" not in _cache:
        _cache["nc"] = _build()
    return _cache["nc"]


def _host_prep(w, b):
    w = np.asarray(w, dtype=np.float32)
    b = np.asarray(b, dtype=np.float32)
    w_pack = np.empty((128, 384), dtype=np.float32)
    off = 0
    for l, (mul, d) in enumerate(IRREPS):
        W = w[off:off + mul * mul].reshape(mul, mul)  # [u, v]
        w_pack[:, l * 128:(l + 1) * 128] = PW * W
        off += mul * mul
    bias_bcast = np.broadcast_to(b[None, :], (128, 128)).copy()
    ident = np.eye(128, dtype=np.float32)
    return w_pack, bias_bcast, ident


def _ensure_ntff_hook():
    """The agent image's antenv lacks axon_hooks; synthesize it from the
    boot package's ctypes NTFF hook so trace=True works."""
    import sys
    import types

    if "antenv.axon_hooks" in sys.modules:
        return
    try:
        from trn_agent_boot.trn_boot import _ntff_profile_via_ctypes

        hook = _ntff_profile_via_ctypes("/opt/axon/libaxon_pjrt.so")
    except Exception:
        hook = None
    mod = types.ModuleType("antenv.axon_hooks")
    state = {"hook": hook}
    mod.get_axon_ntff_profile_hook = lambda: state["hook"]
    mod.set_axon_ntff_profile_hook = lambda h: state.__setitem__("hook", h)
    sys.modules["antenv.axon_hooks"] = mod
    import antenv

    antenv.axon_hooks = mod


def kernel(x, w, b, *, trace=False, trace_cores=None):
    if trace:
        _ensure_ntff_hook()
    x = np.ascontiguousarray(np.asarray(x, dtype=np.float32))
    assert x.shape == (N_NODES, DIM)
    w_pack, bias_bcast, ident = _host_prep(w, b)

    x_pad = np.zeros((PAD_NODES, DIM), dtype=np.float32)
    x_pad[:N_NODES] = x

    in_maps = [
        {
            "x": x_pad[c * SHARD:(c + 1) * SHARD],
            "w": w_pack,
            "bias": bias_bcast,
            "ident": ident,
        }
        for c in range(N_CORES)
    ]
    nc = _get_nc()
    res = run_bass_kernel_spmd(
        nc, in_maps, list(range(N_CORES)), trace=trace, trace_cores=trace_cores
    )
    _cache["last_result"] = res
    y = np.concatenate([res.results[c]["y"] for c in range(N_CORES)], axis=0)
    return y[:N_NODES]
